# revision 1
# baseline (speedup 1.0000x reference)
import sys

sys.path.insert(0, "/opt/trn_rl_repo")

import numpy as np

import concourse.bacc as bacc
import concourse.bass as bass
import concourse.mybir as mybir
import concourse.tile as tile
from concourse.bass_utils import run_bass_kernel_spmd

# Problem shapes (hardcoded per contract)
B = 4
NQ = 2048
NR = 16384
D = 64
K = 16

NCORES = 8
QPC = NQ // 2          # queries per core (each batch split across 2 cores)
NCHUNK = QPC // 128    # query chunks of 128 per core
MMN = 512              # matmul free dim (one PSUM bank of fp32)
GRP = 1024             # candidate block width (2 PSUM banks); top-8 per group
NGRP = NR // GRP       # 16 groups
NCAND = NGRP * 8       # 128 candidates per row

_prog_cache = {}


def _build_program(reps: int = 1):
    if reps in _prog_cache:
        return _prog_cache[reps]

    f32 = mybir.dt.float32
    u32 = mybir.dt.uint32

    nc = bacc.Bacc("TRN2", target_bir_lowering=False, debug=False, num_devices=NCORES)

    # lhsT rows 0..63 = 2*q^T, row 64 = 1.0, row 65 = q2  -> psum = 2qr - r2 - q2 = -d2
    lhs_d = nc.dram_tensor("lhs", [66, QPC], f32, kind="ExternalInput")
    rhs_d = nc.dram_tensor("rhs", [66, NR], f32, kind="ExternalInput")

    outD_d = nc.dram_tensor("outD", [QPC, K], f32, kind="ExternalOutput")
    outP_d = nc.dram_tensor("outP", [QPC, K], u32, kind="ExternalOutput")
    outCI_d = nc.dram_tensor("outCI", [QPC, NCAND], u32, kind="ExternalOutput")

    with tile.TileContext(nc) as tc:
        with (
            tc.tile_pool(name="consts", bufs=1) as cpool,
            tc.tile_pool(name="psum", bufs=8, space="PSUM") as ppool,
            tc.tile_pool(name="stage", bufs=6) as spool,
            tc.tile_pool(name="cands", bufs=3) as candpool,
            tc.tile_pool(name="merge", bufs=2) as mpool,
        ):
            lhs_t = cpool.tile([66, QPC], f32)
            nc.sync.dma_start(lhs_t[:], lhs_d.ap())
            rhs_t = cpool.tile([66, NR], f32)
            nc.sync.dma_start(rhs_t[:], rhs_d.ap())

            for rep in range(reps):
              for c in range(NCHUNK):
                cands_v = candpool.tile([128, NCAND], f32, tag="cv")
                cands_i = candpool.tile([128, NCAND], u32, tag="ci")
                for g in range(NGRP):
                    st = spool.tile([128, GRP], f32, tag="st")
                    for h in range(GRP // MMN):
                        ps = ppool.tile([128, MMN], f32, tag="ps")
                        nc.tensor.matmul(
                            ps[:],
                            lhs_t[:, c * 128:(c + 1) * 128],
                            rhs_t[:, g * GRP + h * MMN:g * GRP + (h + 1) * MMN],
                            start=True,
                            stop=True,
                        )
                        nc.scalar.copy(st[:, h * MMN:(h + 1) * MMN], ps[:])
                    s = g * 8
                    nc.vector.max(cands_v[:, s:s + 8], st[:])
                    nc.vector.max_index(cands_i[:, s:s + 8], cands_v[:, s:s + 8], st[:])

                # merge candidates -> top-16 (values + candidate slots)
                v16 = mpool.tile([128, K], f32, tag="v16")
                p16 = mpool.tile([128, K], u32, tag="p16")
                mr = mpool.tile([128, NCAND], f32, tag="mr")
                nc.vector.max(v16[:, 0:8], cands_v[:])
                nc.vector.max_index(p16[:, 0:8], v16[:, 0:8], cands_v[:])
                nc.vector.match_replace(mr[:], v16[:, 0:8], cands_v[:], -1e30)
                nc.vector.max(v16[:, 8:16], mr[:])
                nc.vector.max_index(p16[:, 8:16], v16[:, 8:16], mr[:])

                # D = sqrt(relu(-v16))
                dsq = mpool.tile([128, K], f32, tag="dsq")
                d16 = mpool.tile([128, K], f32, tag="d16")
                nc.scalar.activation(
                    dsq[:], v16[:], mybir.ActivationFunctionType.Relu, scale=-1.0
                )
                nc.scalar.activation(d16[:], dsq[:], mybir.ActivationFunctionType.Sqrt)

                r0, r1 = c * 128, (c + 1) * 128
                nc.sync.dma_start(outD_d.ap()[r0:r1, :], d16[:])
                nc.sync.dma_start(outP_d.ap()[r0:r1, :], p16[:])
                nc.sync.dma_start(outCI_d.ap()[r0:r1, :], cands_i[:])

    nc.compile()
    _prog_cache[reps] = nc
    return nc


def kernel(ref: np.ndarray, query: np.ndarray):
    ref = np.asarray(ref, dtype=np.float32)
    query = np.asarray(query, dtype=np.float32)

    # host-side operand prep (layout + norms)
    r2 = np.sum(ref * ref, axis=-1)                      # [B, NR]
    q2 = np.sum(query * query, axis=-1)                  # [B, NQ]
    refT = np.ascontiguousarray(ref.transpose(0, 2, 1))  # [B, D, NR]
    qT = np.ascontiguousarray(query.transpose(0, 2, 1))  # [B, D, NQ]

    nc = _build_program()

    in_maps = []
    for core in range(NCORES):
        b, h = core // 2, core % 2
        lhs = np.empty((66, QPC), dtype=np.float32)
        lhs[0:D, :] = 2.0 * qT[b][:, h * QPC:(h + 1) * QPC]
        lhs[D, :] = 1.0
        lhs[D + 1, :] = q2[b, h * QPC:(h + 1) * QPC]
        rhs = np.empty((66, NR), dtype=np.float32)
        rhs[0:D, :] = refT[b]
        rhs[D, :] = -r2[b]
        rhs[D + 1, :] = -1.0
        in_maps.append({"lhs": lhs, "rhs": rhs})

    res = run_bass_kernel_spmd(nc, in_maps, core_ids=list(range(NCORES)))

    Dout = np.empty((B, NQ, K), dtype=np.float32)
    Iout = np.empty((B, NQ, K), dtype=np.int64)
    rows = np.arange(QPC)[:, None]
    for core in range(NCORES):
        b, h = core // 2, core % 2
        r = res.results[core]
        d16 = r["outD"]                      # [QPC, K] f32
        p16 = r["outP"].astype(np.int64)     # [QPC, K] candidate slots
        ci = r["outCI"].astype(np.int64)     # [QPC, NCAND] local idx in group
        gi = ci[rows, p16] + GRP * (p16 >> 3)
        Dout[b, h * QPC:(h + 1) * QPC] = d16
        Iout[b, h * QPC:(h + 1) * QPC] = gi
    return (Dout, Iout)



# revision 6
# speedup vs baseline: 2.1970x; 2.1970x over previous
import sys

sys.path.insert(0, "/opt/trn_rl_repo")

import numpy as np

import concourse.bacc as bacc
import concourse.bass as bass
import concourse.mybir as mybir
import concourse.tile as tile
from concourse.bass_utils import run_bass_kernel_spmd

# Problem shapes (hardcoded per contract)
B = 4
NQ = 2048
NR = 16384
D = 64
K = 16

NCORES = 8
QPC = NQ // 2          # queries per core (each batch split across 2 cores)
NCHUNK = QPC // 128    # query chunks of 128 per core
HALF = 2048            # columns per psum tile (4 banks)
NHALF = NR // HALF     # 8 halves per chunk
MMN = 512              # matmul free dim (one PSUM bank)
TW = 512               # final slot-scan width
NROUND = 3             # top-8 rounds -> 24 slots
NSLOT = 8 * NROUND
SLOTSPAN = 32          # columns covered per slot (4 col-folds x 8 halves)

GAMMA = 327.0
CBIAS = 100.0

_prog_cache = {}


def _build_program(reps: int = 1):
    if reps in _prog_cache:
        return _prog_cache[reps]

    f32 = mybir.dt.float32
    f32r = mybir.dt.float32r
    u16 = mybir.dt.uint16
    u32 = mybir.dt.uint32
    amax = mybir.AluOpType.max

    nc = bacc.Bacc("TRN2", target_bir_lowering=False, debug=False, num_devices=NCORES)

    # psum = gamma*(C - d2) >= 0 for all plausible d2; negatives relu-clamped.
    # lhs rows 0..63 = 2*gamma*q^T, row 64 = -gamma, row 65 = gamma*(C - q2)
    # rhs rows 0..63 = ref^T,       row 64 = r2,     row 65 = 1.0
    lhs_d = nc.dram_tensor("lhs", [66, QPC], f32r, kind="ExternalInput")
    rhs_d = nc.dram_tensor("rhs", [66, NR], f32r, kind="ExternalInput")
    iota_d = nc.dram_tensor("iota", [128, TW], u32, kind="ExternalInput")
    outS_d = nc.dram_tensor("outS", [QPC, NSLOT], u32, kind="ExternalOutput")

    with tile.TileContext(nc) as tc:
        with (
            tc.tile_pool(name="consts", bufs=1) as cpool,
            tc.tile_pool(name="psum", bufs=2, space="PSUM") as ppool,
            tc.tile_pool(name="hbuf", bufs=2) as hpool,
            tc.tile_pool(name="merge", bufs=2) as mpool,
        ):
            lhs_t = cpool.tile([66, QPC], f32r)
            nc.sync.dma_start(lhs_t[:], lhs_d.ap())
            rhs_t = cpool.tile([66, NR], f32r)
            nc.sync.dma_start(rhs_t[:], rhs_d.ap())
            iota_t = cpool.tile([128, TW], u32)
            nc.sync.dma_start(iota_t[:], iota_d.ap())

            for rep in range(reps):
              for c in range(NCHUNK):
                lhs_c = lhs_t[:, c * 128:(c + 1) * 128]
                p1 = [None] * 4
                p2 = [None] * 2
                hprev = None
                for h in range(NHALF):
                    ps = ppool.tile([128, HALF], f32, tag="ps")
                    for i in range(HALF // MMN):
                        nc.tensor.matmul(
                            ps[:, i * MMN:(i + 1) * MMN],
                            lhs_c,
                            rhs_t[:, h * HALF + i * MMN:h * HALF + (i + 1) * MMN],
                            start=True,
                            stop=True,
                        )
                    # convert to u16 coarse keys (monotone in -d2), clamp negatives
                    hbuf = hpool.tile([128, HALF], u16, tag=f"h{h % 2}")
                    nc.scalar.activation(
                        hbuf[:], ps[:], mybir.ActivationFunctionType.Relu
                    )
                    if h % 2 == 1:
                        i1 = h // 2
                        p1t = hpool.tile([128, HALF], u16, tag=f"p1_{i1 % 2}")
                        p1[i1] = p1t
                        nc.vector.tensor_tensor(p1t[:], hprev[:], hbuf[:], amax)
                        if i1 % 2 == 1:
                            i2 = i1 // 2
                            p2t = hpool.tile([128, HALF], u16, tag=f"p2_{i2}")
                            p2[i2] = p2t
                            nc.vector.tensor_tensor(
                                p2t[:], p1[i1 - 1][:], p1[i1][:], amax
                            )
                    hprev = hbuf
                ubuf = hpool.tile([128, HALF], u16, tag="u")
                nc.vector.tensor_tensor(ubuf[:], p2[0][:], p2[1][:], amax)

                # column folds 2048 -> 1024 -> 512
                t1 = mpool.tile([128, 1024], u16, tag="t1")
                nc.vector.tensor_tensor(t1[:], ubuf[:, 0:1024], ubuf[:, 1024:2048], amax)
                tt = mpool.tile([128, TW], u16, tag="tt")
                nc.vector.tensor_tensor(tt[:], t1[:, 0:TW], t1[:, TW:1024], amax)

                # unique u32 keys: value*512 + slot_id  (exact: fits 2^24)
                t32 = mpool.tile([128, TW], u32, tag="t32")
                nc.vector.scalar_tensor_tensor(
                    t32[:], tt[:], 512.0, iota_t[:],
                    mybir.AluOpType.mult, mybir.AluOpType.add,
                )
                # 3 rounds of top-8 slot keys (no max_index needed)
                s24 = mpool.tile([128, NSLOT], u32, tag="s24")
                cur = t32
                for r in range(NROUND):
                    nc.vector.max(s24[:, r * 8:(r + 1) * 8], cur[:])
                    if r + 1 < NROUND:
                        nxt = mpool.tile([128, TW], u32, tag=f"mr_{r}")
                        nc.vector.match_replace(
                            nxt[:], s24[:, r * 8:(r + 1) * 8], cur[:], 0
                        )
                        cur = nxt

                r0, r1 = c * 128, (c + 1) * 128
                nc.sync.dma_start(outS_d.ap()[r0:r1, :], s24[:])

    nc.compile()
    _prog_cache[reps] = nc
    return nc


def kernel(ref: np.ndarray, query: np.ndarray):
    ref = np.asarray(ref, dtype=np.float32)
    query = np.asarray(query, dtype=np.float32)

    r2 = np.sum(ref * ref, axis=-1)                      # [B, NR]
    q2 = np.sum(query * query, axis=-1)                  # [B, NQ]
    refT = np.ascontiguousarray(ref.transpose(0, 2, 1))  # [B, D, NR]
    qT = np.ascontiguousarray(query.transpose(0, 2, 1))  # [B, D, NQ]

    nc = _build_program()

    iota_host = np.broadcast_to(
        np.arange(TW, dtype=np.uint32), (128, TW)
    ).copy()
    in_maps = []
    for core in range(NCORES):
        b, h = core // 2, core % 2
        lhs = np.empty((66, QPC), dtype=np.float32)
        lhs[0:D, :] = (2.0 * GAMMA) * qT[b][:, h * QPC:(h + 1) * QPC]
        lhs[D, :] = -GAMMA
        lhs[D + 1, :] = GAMMA * (CBIAS - q2[b, h * QPC:(h + 1) * QPC])
        rhs = np.empty((66, NR), dtype=np.float32)
        rhs[0:D, :] = refT[b]
        rhs[D, :] = r2[b]
        rhs[D + 1, :] = 1.0
        in_maps.append({"lhs": lhs, "rhs": rhs, "iota": iota_host})

    res = run_bass_kernel_spmd(nc, in_maps, core_ids=list(range(NCORES)))

    # host-side FAISS-style merge: decode candidate slots, exact-rescore, top-16
    span = (512 * np.arange(4)[:, None] + HALF * np.arange(NHALF)[None, :]).ravel()
    Dout = np.empty((B, NQ, K), dtype=np.float32)
    Iout = np.empty((B, NQ, K), dtype=np.int64)
    for core in range(NCORES):
        b, h = core // 2, core % 2
        slots = (res.results[core]["outS"] & 511).astype(np.int64)  # [QPC, NSLOT]
        cols = (slots[:, :, None] + span[None, None, :]).reshape(QPC, -1)
        cols.sort(axis=1)
        dup = np.zeros(cols.shape, dtype=bool)
        dup[:, 1:] = cols[:, 1:] == cols[:, :-1]
        q_core = query[b][h * QPC:(h + 1) * QPC]                 # [QPC, D]
        q2_core = q2[b, h * QPC:(h + 1) * QPC]
        for s in range(0, QPC, 128):
            cs = cols[s:s + 128]                                 # [128, NCAND]
            g = ref[b][cs]                                       # [128, NCAND, D]
            dots = np.einsum('qd,qkd->qk', q_core[s:s + 128], g, optimize=True)
            d2 = q2_core[s:s + 128, None] + r2[b][cs] - 2.0 * dots
            d2 = np.maximum(d2, 0.0)
            d2[dup[s:s + 128]] = np.inf
            order = np.argsort(d2, axis=1, kind='stable')[:, :K]
            rows = np.arange(128)[:, None]
            Dout[b, h * QPC + s:h * QPC + s + 128] = np.sqrt(d2[rows, order])
            Iout[b, h * QPC + s:h * QPC + s + 128] = cs[rows, order]
    return (Dout, Iout)


# revision 7
# speedup vs baseline: 2.3596x; 1.0740x over previous
import sys

sys.path.insert(0, "/opt/trn_rl_repo")

import numpy as np

import concourse.bacc as bacc
import concourse.bass as bass
import concourse.mybir as mybir
import concourse.tile as tile
from concourse.bass_utils import run_bass_kernel_spmd

# Problem shapes (hardcoded per contract)
B = 4
NQ = 2048
NR = 16384
D = 64
K = 16

NCORES = 8
QPC = NQ // 2          # queries per core (each batch split across 2 cores)
NCHUNK = QPC // 128    # query chunks of 128 per core
HALF = 2048            # columns per psum tile (4 banks)
NHALF = NR // HALF     # 8 halves per chunk
MMN = 512              # matmul free dim (one PSUM bank)
TW = 512               # final slot-scan width
NROUND = 3             # top-8 rounds -> 24 slots
NSLOT = 8 * NROUND
SLOTSPAN = 32          # columns covered per slot (4 col-folds x 8 halves)

GAMMA = 327.0
CBIAS = 100.0

_prog_cache = {}


def _build_program(reps: int = 1):
    if reps in _prog_cache:
        return _prog_cache[reps]

    f32 = mybir.dt.float32
    f32r = mybir.dt.float32r
    u16 = mybir.dt.uint16
    u32 = mybir.dt.uint32
    amax = mybir.AluOpType.max

    nc = bacc.Bacc("TRN2", target_bir_lowering=False, debug=False, num_devices=NCORES)

    # psum = gamma*(C - d2) >= 0 for all plausible d2; negatives relu-clamped.
    # lhs rows 0..63 = 2*gamma*q^T, row 64 = -gamma, row 65 = gamma*(C - q2)
    # rhs rows 0..63 = ref^T,       row 64 = r2,     row 65 = 1.0
    lhs_d = nc.dram_tensor("lhs", [66, QPC], f32r, kind="ExternalInput")
    rhs_d = nc.dram_tensor("rhs", [66, NR], f32r, kind="ExternalInput")
    iota_d = nc.dram_tensor("iota", [128, TW], u32, kind="ExternalInput")
    outS_d = nc.dram_tensor("outS", [QPC, NSLOT], u32, kind="ExternalOutput")

    with tile.TileContext(nc) as tc:
        with (
            tc.tile_pool(name="consts", bufs=1) as cpool,
            tc.tile_pool(name="psum", bufs=2, space="PSUM") as ppool,
            tc.tile_pool(name="hbuf", bufs=2) as hpool,
            tc.tile_pool(name="merge", bufs=2) as mpool,
        ):
            lhs_t = cpool.tile([66, QPC], f32r)
            nc.sync.dma_start(lhs_t[:], lhs_d.ap())
            rhs_tiles = []
            for hh in range(NHALF):
                rt = cpool.tile([66, HALF], f32r, tag=f"rhs{hh}")
                nc.sync.dma_start(rt[:], rhs_d.ap()[:, hh * HALF:(hh + 1) * HALF])
                rhs_tiles.append(rt)
            iota_t = cpool.tile([128, TW], u32)
            nc.sync.dma_start(iota_t[:], iota_d.ap())

            for rep in range(reps):
              for c in range(NCHUNK):
                lhs_c = lhs_t[:, c * 128:(c + 1) * 128]
                p1 = [None] * 4
                p2 = [None] * 2
                hprev = None
                for h in range(NHALF):
                    ps = ppool.tile([128, HALF], f32, tag="ps")
                    for i in range(HALF // MMN):
                        nc.tensor.matmul(
                            ps[:, i * MMN:(i + 1) * MMN],
                            lhs_c,
                            rhs_tiles[h][:, i * MMN:(i + 1) * MMN],
                            start=True,
                            stop=True,
                        )
                    # convert to u16 coarse keys (monotone in -d2), clamp negatives
                    hbuf = hpool.tile([128, HALF], u16, tag=f"h{h % 2}")
                    nc.scalar.activation(
                        hbuf[:], ps[:], mybir.ActivationFunctionType.Relu
                    )
                    if h % 2 == 1:
                        i1 = h // 2
                        p1t = hpool.tile([128, HALF], u16, tag=f"p1_{i1 % 2}")
                        p1[i1] = p1t
                        nc.vector.tensor_tensor(p1t[:], hprev[:], hbuf[:], amax)
                        if i1 % 2 == 1:
                            i2 = i1 // 2
                            p2t = hpool.tile([128, HALF], u16, tag=f"p2_{i2}")
                            p2[i2] = p2t
                            nc.vector.tensor_tensor(
                                p2t[:], p1[i1 - 1][:], p1[i1][:], amax
                            )
                    hprev = hbuf
                ubuf = hpool.tile([128, HALF], u16, tag="u")
                nc.vector.tensor_tensor(ubuf[:], p2[0][:], p2[1][:], amax)

                # column folds 2048 -> 1024 -> 512
                t1 = mpool.tile([128, 1024], u16, tag="t1")
                nc.vector.tensor_tensor(t1[:], ubuf[:, 0:1024], ubuf[:, 1024:2048], amax)
                tt = mpool.tile([128, TW], u16, tag="tt")
                nc.vector.tensor_tensor(tt[:], t1[:, 0:TW], t1[:, TW:1024], amax)

                # unique u32 keys: value*512 + slot_id  (exact: fits 2^24)
                t32 = mpool.tile([128, TW], u32, tag="t32")
                nc.vector.scalar_tensor_tensor(
                    t32[:], tt[:], 512.0, iota_t[:],
                    mybir.AluOpType.mult, mybir.AluOpType.add,
                )
                # 3 rounds of top-8 slot keys (no max_index needed)
                s24 = mpool.tile([128, NSLOT], u32, tag="s24")
                cur = t32
                for r in range(NROUND):
                    nc.vector.max(s24[:, r * 8:(r + 1) * 8], cur[:])
                    if r + 1 < NROUND:
                        nxt = mpool.tile([128, TW], u32, tag=f"mr_{r}")
                        nc.vector.match_replace(
                            nxt[:], s24[:, r * 8:(r + 1) * 8], cur[:], 0
                        )
                        cur = nxt

                r0, r1 = c * 128, (c + 1) * 128
                nc.sync.dma_start(outS_d.ap()[r0:r1, :], s24[:])

    nc.compile()
    _prog_cache[reps] = nc
    return nc


def kernel(ref: np.ndarray, query: np.ndarray):
    ref = np.asarray(ref, dtype=np.float32)
    query = np.asarray(query, dtype=np.float32)

    r2 = np.sum(ref * ref, axis=-1)                      # [B, NR]
    q2 = np.sum(query * query, axis=-1)                  # [B, NQ]
    refT = np.ascontiguousarray(ref.transpose(0, 2, 1))  # [B, D, NR]
    qT = np.ascontiguousarray(query.transpose(0, 2, 1))  # [B, D, NQ]

    nc = _build_program()

    iota_host = np.broadcast_to(
        np.arange(TW, dtype=np.uint32), (128, TW)
    ).copy()
    in_maps = []
    for core in range(NCORES):
        b, h = core // 2, core % 2
        lhs = np.empty((66, QPC), dtype=np.float32)
        lhs[0:D, :] = (2.0 * GAMMA) * qT[b][:, h * QPC:(h + 1) * QPC]
        lhs[D, :] = -GAMMA
        lhs[D + 1, :] = GAMMA * (CBIAS - q2[b, h * QPC:(h + 1) * QPC])
        rhs = np.empty((66, NR), dtype=np.float32)
        rhs[0:D, :] = refT[b]
        rhs[D, :] = r2[b]
        rhs[D + 1, :] = 1.0
        in_maps.append({"lhs": lhs, "rhs": rhs, "iota": iota_host})

    res = run_bass_kernel_spmd(nc, in_maps, core_ids=list(range(NCORES)))

    # host-side FAISS-style merge: decode candidate slots, exact-rescore, top-16
    span = (512 * np.arange(4)[:, None] + HALF * np.arange(NHALF)[None, :]).ravel()
    Dout = np.empty((B, NQ, K), dtype=np.float32)
    Iout = np.empty((B, NQ, K), dtype=np.int64)
    for core in range(NCORES):
        b, h = core // 2, core % 2
        slots = (res.results[core]["outS"] & 511).astype(np.int64)  # [QPC, NSLOT]
        cols = (slots[:, :, None] + span[None, None, :]).reshape(QPC, -1)
        cols.sort(axis=1)
        dup = np.zeros(cols.shape, dtype=bool)
        dup[:, 1:] = cols[:, 1:] == cols[:, :-1]
        q_core = query[b][h * QPC:(h + 1) * QPC]                 # [QPC, D]
        q2_core = q2[b, h * QPC:(h + 1) * QPC]
        for s in range(0, QPC, 128):
            cs = cols[s:s + 128]                                 # [128, NCAND]
            g = ref[b][cs]                                       # [128, NCAND, D]
            dots = np.einsum('qd,qkd->qk', q_core[s:s + 128], g, optimize=True)
            d2 = q2_core[s:s + 128, None] + r2[b][cs] - 2.0 * dots
            d2 = np.maximum(d2, 0.0)
            d2[dup[s:s + 128]] = np.inf
            order = np.argsort(d2, axis=1, kind='stable')[:, :K]
            rows = np.arange(128)[:, None]
            Dout[b, h * QPC + s:h * QPC + s + 128] = np.sqrt(d2[rows, order])
            Iout[b, h * QPC + s:h * QPC + s + 128] = cs[rows, order]
    return (Dout, Iout)
